# revision 11
# baseline (speedup 1.0000x reference)
"""Trainium2 Bass kernel for nn_ContrastiveEmbeddingLoss.

Reference computation (N=8192, D=128, margin=1.0):
    d[i,j]  = ||x_i - x_j||^2          (clamped at 0)
    same    = (y_i == y_j)
    loss    = mean((1-same)*d + same*relu(margin - d))

Algebraic decomposition used here:
    loss_sum = sum_ij d  -  sum_same d  +  sum_same relu(1 - d)

The first two terms are exact O(N*D) sums-of-moments computed on host in
float64. Only sum_same relu(1 - d) needs pairwise work, and `same` is
nonzero only within a class. The host sorts rows by class into 128-wide
slots (13 slots/core x 8 cores = 104 >= number of classes).

Device math per slot (DoubleRow fp8 matmul, effective K = 130 in one
instruction): with u_i = fp8(sqrt2 * x_i) and s_i = ||u_i||^2 / 2,

    psum[p,f] = <u_p, u_f> + 2*(1 - s_f)/2 + (s_p/2)*(-2) = 1 - d[p,f]

so a single K=65x2 DoubleRow matmul per slot produces 1-d directly
(zero-padded slots give psum=0).  The gram rows live in k-tile pairs
(dims 0-64 in tile 0, dims 65-127 + two augmentation rows in tile 1);
DoubleRow contracts over both tiles: out = sum_i L[:,i,:].T @ R[:,i,:].

relu + per-partition sum: two ScalarE activations with accum (psa slots
0-7, psb slots 8-12), balanced so act1 overlaps the matmul tail. 13
single matmuls replace the baseline's 26; each streams its 128 columns
through the PE once instead of twice.

SBUF layout is slot-interleaved [KH, 13, 2, 128] so each slot's lhsT/rhs
view [KH, 2, 128] has a contiguous [2, 128] free block (DoubleRow
hardware layout).
"""

import numpy as np
import ml_dtypes

N, D = 8192, 128
MARGIN = 1.0
NCORES = 8
SLOT = 128               # columns per class slot (max class size supported)
SLOTS_PER_CORE = 13      # 8*13 = 104 slots >= 100 classes
W = SLOTS_PER_CORE * SLOT
KH = 65                  # partition rows: 2 k-tiles of 65 = 130 effective rows
_FP8 = ml_dtypes.float8_e4m3
_NC = None

# PSUM regions: psa covers slots 0..5, psb slots 6..12 (both ScalarE).
# Balanced so act1 (after psa closes mid-matmul-phase) finishes right as
# the matmuls end, minimizing the serial act1+act2 tail.
NSLOTS_A = 6
NSLOTS_B = SLOTS_PER_CORE - NSLOTS_A
CA = NSLOTS_A * SLOT     # 768
CB = NSLOTS_B * SLOT     # 896


def _build_nc():
    """Raw bacc program: 4 input DMAs on 3 engines, 13 DoubleRow matmuls,
    two relu+accumulate activations on ScalarE, one output DMA of the two
    per-partition sums."""
    import concourse.bacc as bacc
    import concourse.mybir as mybir

    nc = bacc.Bacc(None, target_bir_lowering=False)
    fp8 = mybir.dt.float8e4
    f32 = mybir.dt.float32
    bf16 = mybir.dt.bfloat16
    DR = mybir.MatmulPerfMode.DoubleRow
    S = SLOTS_PER_CORE

    l8 = nc.declare_dram_parameter("l8", [KH, S, 2, SLOT], fp8, isOutput=False)
    r8 = nc.declare_dram_parameter("r8", [KH, S, 2, SLOT], fp8, isOutput=False)
    acc = nc.declare_dram_parameter("acc", [D, 2], f32, isOutput=True)

    with (
        nc.sbuf_tensor("l8t", [KH, S, 2, SLOT], fp8) as l8t,
        nc.sbuf_tensor("r8t", [KH, S, 2, SLOT], fp8) as r8t,
        nc.sbuf_tensor("accst", [D, 2], f32) as accst,
        nc.sbuf_tensor("v1a", [D, CA], bf16) as v1a,
        nc.sbuf_tensor("v1b", [D, CB], bf16) as v1b,
        nc.psum_tensor("psa", [D, CA], f32) as psa,
        nc.psum_tensor("psb", [D, CB], f32) as psb,
        nc.semaphore("s_a") as s_a,
        nc.semaphore("s_b") as s_b,
        nc.semaphore("s_c") as s_c,
        nc.semaphore("s_d") as s_d,
        nc.semaphore("s_e") as s_e,
        nc.semaphore("s_mm") as s_mm,
        nc.semaphore("s_act") as s_act,
        nc.semaphore("s_out") as s_out,
        nc.Block() as block,
    ):
        def pslot(s):
            if s < NSLOTS_A:
                return psa[:, s * SLOT : (s + 1) * SLOT]
            s -= NSLOTS_A
            return psb[:, s * SLOT : (s + 1) * SLOT]

        # 6 DMAs on the 3 DMA-capable engines, interleaved so each matmul
        # chunk-group's two tensors land on different queues: slots 0-4
        # (sync: l8, scalar: r8), slots 5-8 (gpsimd both), slots 9-12
        # (sync: l8, gpsimd: r8).
        @block.sync
        def _(sync):
            sync.dma_start(l8t[:, 0:5], l8[:, 0:5]).then_inc(s_a, 16)
            sync.dma_start(l8t[:, 9:S], l8[:, 9:S]).then_inc(s_d, 16)
            sync.wait_ge(s_act, 2)
            sync.dma_start(acc[:], accst[:]).then_inc(s_out, 16)

        @block.gpsimd
        def _(gpsimd):
            gpsimd.dma_start(r8t[:, 5:9], r8[:, 5:9]).then_inc(s_c, 16)
            gpsimd.dma_start(l8t[:, 5:9], l8[:, 5:9]).then_inc(s_c, 16)
            gpsimd.dma_start(r8t[:, 9:S], r8[:, 9:S]).then_inc(s_e, 16)

        def mm(s):
            return nc.tensor.matmul(
                pslot(s), l8t[:, s], r8t[:, s],
                start=True, stop=True, perf_mode=DR,
            )

        @block.tensor
        def _(tensor):
            tensor.wait_ge(s_a, 16)
            tensor.wait_ge(s_b, 16)
            for s in range(5):
                mm(s)
            tensor.wait_ge(s_c, 32)
            mm(NSLOTS_A - 1).then_inc(s_mm, 1)   # slot 5 closes psa
            for s in range(NSLOTS_A, 9):
                mm(s)
            tensor.wait_ge(s_d, 16)
            tensor.wait_ge(s_e, 16)
            for s in range(9, S - 1):
                mm(s)
            mm(S - 1).then_inc(s_mm, 1)          # psb complete

        @block.scalar
        def _(scalar):
            scalar.dma_start(r8t[:, 0:5], r8[:, 0:5]).then_inc(s_b, 16)
            scalar.wait_ge(s_mm, 1)
            nc.scalar.activation(
                v1a[:], psa[:], mybir.ActivationFunctionType.Relu,
                bias=0.0, scale=1.0, accum_out=accst[:, 0:1],
            ).then_inc(s_act, 1)
            scalar.wait_ge(s_mm, 2)
            nc.scalar.activation(
                v1b[:], psb[:], mybir.ActivationFunctionType.Relu,
                bias=0.0, scale=1.0, accum_out=accst[:, 1:2],
            ).then_inc(s_act, 1)

    nc.finalize()
    return nc


def _get_nc():
    global _NC
    if _NC is None:
        _NC = _build_nc()
    return _NC


def _prepare_inputs(x_np, y_np):
    """Host-side packing + exact fp64 moment sums.

    Returns (in_maps, sum_d_all, sum_d_same)."""
    x64 = x_np.astype(np.float64)
    sq64 = np.einsum("ij,ij->i", x64, x64)
    s_all = x64.sum(0)
    sum_d_all = 2.0 * N * sq64.sum() - 2.0 * float(s_all @ s_all)

    order = np.argsort(y_np, kind="stable")
    uniq, counts = np.unique(y_np, return_counts=True)
    assert len(uniq) <= NCORES * SLOTS_PER_CORE, "too many classes for slots"
    assert counts.max() <= SLOT, "class larger than one slot"

    # fp8-quantized rows: u = fp8(sqrt2 * x); s = ||u||^2 / 2 (fp32 from u)
    root2 = np.float32(np.sqrt(2.0))
    U = (root2 * x_np).astype(_FP8)
    Uf = U.astype(np.float32)
    s_q = 0.5 * np.einsum("ij,ij->i", Uf, Uf)

    # Effective contraction rows (k, tile): tile 0 = u dims 0..64;
    # tile 1 = u dims 65..127 then [L=2, R=(1-s)/2] and [L=s/2, R=-2].
    L8 = np.zeros((NCORES, KH, SLOTS_PER_CORE, 2, SLOT), np.float32)
    R8 = np.zeros((NCORES, KH, SLOTS_PER_CORE, 2, SLOT), np.float32)
    sum_d_same = 0.0
    pos = 0
    for ci, n_c in enumerate(counts):
        idx = order[pos : pos + n_c]
        pos += n_c
        core, ls = divmod(ci, SLOTS_PER_CORE)
        cs = slice(0, n_c)
        ut = Uf[idx].T                      # [128, n_c]
        L8[core][:, ls, 0, cs] = ut[0:KH]
        L8[core][0 : D - KH, ls, 1, cs] = ut[KH:D]
        R8[core][:, ls, 0, cs] = ut[0:KH]
        R8[core][0 : D - KH, ls, 1, cs] = ut[KH:D]
        L8[core][KH - 2, ls, 1, cs] = 2.0
        R8[core][KH - 2, ls, 1, cs] = 0.5 * (1.0 - s_q[idx])
        L8[core][KH - 1, ls, 1, cs] = 0.5 * s_q[idx]
        R8[core][KH - 1, ls, 1, cs] = -2.0
        sc = x64[idx].sum(0)
        sum_d_same += 2.0 * n_c * sq64[idx].sum() - 2.0 * float(sc @ sc)

    in_maps = [
        {
            "l8": np.ascontiguousarray(L8[i]).astype(_FP8),
            "r8": np.ascontiguousarray(R8[i]).astype(_FP8),
        }
        for i in range(NCORES)
    ]
    return in_maps, sum_d_all, sum_d_same


def _run_device(in_maps, trace=False):
    from concourse.bass_utils import run_bass_kernel_spmd

    return run_bass_kernel_spmd(
        _get_nc(), in_maps, core_ids=list(range(NCORES)), trace=trace
    )


def kernel(x, y):
    x_np = np.asarray(x, dtype=np.float32).reshape(N, D)
    y_np = np.asarray(y).astype(np.int64).ravel()

    in_maps, sum_d_all, sum_d_same = _prepare_inputs(x_np, y_np)
    res = _run_device(in_maps)
    hinge = sum(float(r["acc"].astype(np.float64).sum()) for r in res.results)

    loss = (sum_d_all - sum_d_same + hinge) / (float(N) * float(N))
    return np.float32(loss)
